# revision 1
# baseline (speedup 1.0000x reference)
"""Trainium2 Bass kernel for DepthConsistencyLoss.

kernel(points, densities, depth_gt) -> np.float32 scalar loss.

8-core SPMD: each core projects a ~1M-point shard of one image (2 cores
per image), accumulates weighted depth / weight histograms in PSUM via
fp16 one-hot matmuls (bin v*256+u -> PSUM[v>>1, (v&1)*256+u]), the
[2,4,256,256] maps are AllReduce-summed across cores, then every core
computes the divide/normalize/SSIM/L1 stages identically and emits the
scalar.
"""

import numpy as np

import concourse.bass as bass
import concourse.tile as tile
from concourse import bacc, mybir
from concourse.bass_utils import run_bass_kernel_spmd

F32 = mybir.dt.float32
F16 = mybir.dt.float16
I32 = mybir.dt.int32
I16 = mybir.dt.int16
ALU = mybir.AluOpType
ACTF = mybir.ActivationFunctionType
AX = mybir.AxisListType

NCORES = 8
B = 4
H = W = 256
HW = H * W
MIN_DEPTH = 0.1
MAX_DEPTH = 10.0
SSIM_C1 = 0.01 ** 2
SSIM_C2 = 0.03 ** 2


def _floor_pos(nc, pool, x, C, tag):
    """floor(x) for x >= 0, robust to convert rounding mode (HW rounds to
    nearest, sim truncates): r = cvt(x); r -= (r > x)."""
    ri = pool.tile([128, C], I32, tag=f"{tag}_ri")
    nc.vector.tensor_copy(ri[:], x[:])
    rf = pool.tile([128, C], F32, tag=f"{tag}_rf")
    nc.vector.tensor_copy(rf[:], ri[:])
    gt = pool.tile([128, C], F32, tag=f"{tag}_gt")
    nc.vector.tensor_tensor(gt[:], rf[:], x[:], ALU.is_gt)
    out = pool.tile([128, C], F32, tag=f"{tag}_out")
    nc.vector.tensor_tensor(out[:], rf[:], gt[:], ALU.subtract)
    return out


def build_nc(chunks_per_tile=512, tiles=16, debug_maps=False, lhs_on_gpsimd=False, no_collective=False, num_devices=NCORES, ablate=(), hist_tiles=None):
    C, T = chunks_per_tile, tiles

    nc = bacc.Bacc("TRN2", target_bir_lowering=False, debug=False,
                   num_devices=num_devices)

    x_in = nc.dram_tensor("x_in", [128, C * T], F32, kind="ExternalInput")
    y_in = nc.dram_tensor("y_in", [128, C * T], F32, kind="ExternalInput")
    z_in = nc.dram_tensor("z_in", [128, C * T], F32, kind="ExternalInput")
    d_in = nc.dram_tensor("d_in", [128, C * T], F32, kind="ExternalInput")
    gt_in = nc.dram_tensor("gt_in", [B, 2, 128, 256], F32, kind="ExternalInput")
    msk_in = nc.dram_tensor("msk_in", [128, B], F32, kind="ExternalInput")
    out_scalar = nc.dram_tensor("out_scalar", [1, 1], F32, kind="ExternalOutput")
    if debug_maps:
        out_zw = nc.dram_tensor("out_zw", [B, 128, 512], F32, kind="ExternalOutput")
        out_wm = nc.dram_tensor("out_wm", [B, 128, 512], F32, kind="ExternalOutput")

    red_in = nc.dram_tensor("red_in", [2, B, 128, 512], F32)
    red_out = nc.dram_tensor("red_out", [2, B, 128, 512], F32)

    lhs_eng = nc.gpsimd if lhs_on_gpsimd else nc.vector

    with tile.TileContext(nc) as tc:
        with (
            tc.tile_pool(name="const", bufs=1) as cpool,
            tc.tile_pool(name="psum", bufs=1, space="PSUM") as psum,
        ):
            hist_pools = (
                tc.tile_pool(name="pts", bufs=2),
                tc.tile_pool(name="work", bufs=2),
                tc.tile_pool(name="workt", bufs=1),
                tc.tile_pool(name="oh", bufs=4),
                tc.tile_pool(name="mid", bufs=1),
            )
            pts = hist_pools[0].__enter__()
            pool = hist_pools[1].__enter__()
            workt = hist_pools[2].__enter__()
            ohp = hist_pools[3].__enter__()
            midp = hist_pools[4].__enter__()
            # ---------- constants ----------
            io512i = cpool.tile([128, 512], I16)
            nc.gpsimd.iota(io512i[:], pattern=[[1, 512]], base=0, channel_multiplier=0)
            io512 = cpool.tile([128, 512], F16)
            nc.vector.tensor_copy(io512[:], io512i[:])
            io128i = cpool.tile([128, 128], I16)
            nc.gpsimd.iota(io128i[:], pattern=[[1, 128]], base=0, channel_multiplier=0)
            io128 = cpool.tile([128, 128], F16)
            nc.vector.tensor_copy(io128[:], io128i[:])

            bands = {}
            for base in (0, 128, -128):
                di = cpool.tile([128, 128], I16, tag=f"bandi_{base}")
                nc.gpsimd.iota(di[:], pattern=[[1, 128]], base=base, channel_multiplier=-1)
                df = cpool.tile([128, 128], F32, tag=f"bandf_{base}")
                nc.vector.tensor_copy(df[:], di[:])
                c1t = cpool.tile([128, 128], F32, tag=f"bandc1_{base}")
                nc.vector.tensor_scalar(c1t[:], df[:], -5.0, None, ALU.is_ge)
                c2t = cpool.tile([128, 128], F32, tag=f"bandc2_{base}")
                nc.vector.tensor_scalar(c2t[:], df[:], 5.0, None, ALU.is_le)
                bt = cpool.tile([128, 128], F32, tag=f"band_{base}")
                nc.vector.tensor_tensor(bt[:], c1t[:], c2t[:], ALU.mult)
                bands[base] = bt
            Bv00, Bv01, Bv10 = bands[0], bands[128], bands[-128]

            msk = cpool.tile([128, B], F32)
            nc.sync.dma_start(msk[:], msk_in[:, :])
            cBIG = cpool.tile([128, 256], F32, tag="cBIG")
            nc.vector.memset(cBIG[:], 1e30)
            cNBIG = cpool.tile([128, 256], F32, tag="cNBIG")
            nc.vector.memset(cNBIG[:], -1e30)
            ones1 = cpool.tile([1, 128], F32, tag="ones1")
            nc.vector.memset(ones1[:], 1.0)

            # ---------- histogram accumulation ----------
            acc_zw = psum.tile([128, 512], F32)
            acc_wm = psum.tile([128, 512], F32)
            nc.vector.memset(acc_zw[:], 0.0)
            nc.vector.memset(acc_wm[:], 0.0)
            zlhs = cpool.tile([128, 128], F16, tag="zlhs")
            nc.vector.memset(zlhs[:], 0.0)
            zrhs = cpool.tile([128, 512], F16, tag="zrhs")
            nc.vector.memset(zrhs[:], 0.0)
            for t in range(T if hist_tiles is None else hist_tiles):
                xt = pts.tile([128, C], F32, tag="xt")
                yt = pts.tile([128, C], F32, tag="yt")
                zt = pts.tile([128, C], F32, tag="zt")
                dt = pts.tile([128, C], F32, tag="dt")
                sl = slice(t * C, (t + 1) * C)
                nc.sync.dma_start(xt[:], x_in[:, sl])
                nc.sync.dma_start(yt[:], y_in[:, sl])
                nc.sync.dma_start(zt[:], z_in[:, sl])
                nc.sync.dma_start(dt[:], d_in[:, sl])

                zs = pool.tile([128, C], F32, tag="zs")
                nc.vector.tensor_scalar(zs[:], zt[:], MIN_DEPTH, None, ALU.max)
                rz = workt.tile([128, C], F32, tag="rz")
                scr = workt.tile([128, C], F32, tag="scr")
                nc.vector.reciprocal_approx_accurate(rz[:], zs[:], scr[:])
                u0 = workt.tile([128, C], F32, tag="u0")
                nc.vector.scalar_tensor_tensor(u0[:], xt[:], 256.0, rz[:], ALU.mult, ALU.mult)
                v0 = workt.tile([128, C], F32, tag="v0")
                nc.vector.scalar_tensor_tensor(v0[:], yt[:], 256.0, rz[:], ALU.mult, ALU.mult)

                mx = workt.tile([128, C], F32, tag="mx")
                nc.vector.tensor_tensor(mx[:], u0[:], v0[:], ALU.max)
                mn = workt.tile([128, C], F32, tag="mn")
                nc.vector.tensor_tensor(mn[:], u0[:], v0[:], ALU.min)
                cv = workt.tile([128, C], F32, tag="cv")
                nc.vector.tensor_scalar(cv[:], mx[:], 128.0, None, ALU.is_lt)
                cv2 = workt.tile([128, C], F32, tag="cv2")
                nc.vector.tensor_scalar(cv2[:], mn[:], -128.0, None, ALU.is_ge)
                cz = workt.tile([128, C], F32, tag="cz")
                nc.vector.tensor_scalar(cz[:], zt[:], MIN_DEPTH, None, ALU.is_gt)
                sg = workt.tile([128, C], F32, tag="sg")
                nc.scalar.activation(sg[:], dt[:], ACTF.Sigmoid)
                vv = workt.tile([128, C], F32, tag="vv")
                nc.vector.tensor_tensor(vv[:], cv[:], cv2[:], ALU.mult)
                vz = workt.tile([128, C], F32, tag="vz")
                nc.vector.tensor_tensor(vz[:], vv[:], cz[:], ALU.mult)
                weff = pool.tile([128, C], F32, tag="weff")
                nc.vector.tensor_tensor(weff[:], vz[:], sg[:], ALU.mult)

                ucl = workt.tile([128, C], F32, tag="ucl")
                nc.vector.tensor_scalar(ucl[:], u0[:], 127.5, -128.0, ALU.min, ALU.max)
                up = workt.tile([128, C], F32, tag="up")
                nc.vector.tensor_scalar(up[:], ucl[:], 128.0, None, ALU.add)
                vcl = workt.tile([128, C], F32, tag="vcl")
                nc.vector.tensor_scalar(vcl[:], v0[:], 127.5, -128.0, ALU.min, ALU.max)
                vp = workt.tile([128, C], F32, tag="vp")
                nc.vector.tensor_scalar(vp[:], vcl[:], 128.0, None, ALU.add)
                uf = _floor_pos(nc, workt, up, C, "uf")
                vf = _floor_pos(nc, workt, vp, C, "vf")

                vi = workt.tile([128, C], I32, tag="vi")
                nc.vector.tensor_copy(vi[:], vf[:])
                hii = workt.tile([128, C], I32, tag="hii")
                nc.vector.tensor_scalar(hii[:], vi[:], 1, None, ALU.arith_shift_right)
                hif = pool.tile([128, C], F32, tag="hif")
                nc.vector.tensor_copy(hif[:], hii[:])
                pri = workt.tile([128, C], I32, tag="pri")
                nc.vector.tensor_scalar(pri[:], vi[:], 1, None, ALU.bitwise_and)
                prf = workt.tile([128, C], F32, tag="prf")
                nc.vector.tensor_copy(prf[:], pri[:])
                lof = pool.tile([128, C], F32, tag="lof")
                nc.vector.scalar_tensor_tensor(lof[:], prf[:], 256.0, uf[:], ALU.mult, ALU.add)

                U = min(64, C)
                with tc.For_i(0, C, U) as iv:
                    lofb = ohp.tile([128, U], F32, tag="lofb")
                    nc.vector.tensor_copy(lofb[:], lof[:, bass.ds(iv, U)])
                    weffb = ohp.tile([128, U], F32, tag="weffb")
                    nc.vector.tensor_copy(weffb[:], weff[:, bass.ds(iv, U)])
                    hifb = ohp.tile([128, U], F32, tag="hifb")
                    nc.gpsimd.tensor_copy(hifb[:], hif[:, bass.ds(iv, U)])
                    zsb = ohp.tile([128, U], F32, tag="zsb")
                    nc.gpsimd.tensor_copy(zsb[:], zs[:, bass.ds(iv, U)])
                    for u in range(U):
                        rhs = ohp.tile([128, 512], F16, tag="rhs")
                        if "rhs" not in ablate:
                            nc.vector.tensor_scalar(rhs[:], io512[:], lofb[:, u:u + 1],
                                                    weffb[:, u:u + 1], ALU.is_equal, ALU.mult)
                        lhsA = ohp.tile([128, 128], F16, tag="lhsA")
                        lhsZ = ohp.tile([128, 128], F16, tag="lhsZ")
                        if "lhs" not in ablate:
                            lhs_eng.tensor_scalar(lhsA[:], io128[:], hifb[:, u:u + 1],
                                                  None, ALU.is_equal)
                            lhs_eng.tensor_scalar(lhsZ[:], io128[:], hifb[:, u:u + 1],
                                                  zsb[:, u:u + 1], ALU.is_equal, ALU.mult)
                        if "mm" not in ablate:
                            rr = zrhs if "rhs" in ablate else rhs
                            lA = zlhs if "lhs" in ablate else lhsA
                            lZ = zlhs if "lhs" in ablate else lhsZ
                            nc.tensor.matmul(acc_wm[:], lA[:], rr[:], start=False,
                                             stop=False, skip_group_check=True)
                            nc.tensor.matmul(acc_zw[:], lZ[:], rr[:], start=False,
                                             stop=False, skip_group_check=True)
            nc.tensor.matmul(acc_wm[:], zlhs[:], zrhs[:], start=False, stop=True,
                             skip_group_check=True)
            nc.tensor.matmul(acc_zw[:], zlhs[:], zrhs[:], start=False, stop=True,
                             skip_group_check=True)

            # ---------- masked placement + AllReduce ----------
            sb_zw = midp.tile([128, 512], F32, tag="sb_zw")
            nc.vector.tensor_copy(sb_zw[:], acc_zw[:])
            sb_wm = midp.tile([128, 512], F32, tag="sb_wm")
            nc.vector.tensor_copy(sb_wm[:], acc_wm[:])
            for b in range(B):
                mzw = midp.tile([128, 512], F32, tag="mzw", bufs=2)
                nc.vector.tensor_scalar(mzw[:], sb_zw[:], msk[:, b:b + 1], None, ALU.mult)
                nc.sync.dma_start(red_in[0, b], mzw[:])
                mwm = midp.tile([128, 512], F32, tag="mwm", bufs=2)
                nc.vector.tensor_scalar(mwm[:], sb_wm[:], msk[:, b:b + 1], None, ALU.mult)
                nc.sync.dma_start(red_in[1, b], mwm[:])

            if no_collective:
                nc.sync.dma_start(red_out[:, :, :, :], red_in[:, :, :, :])
            else:
                nc.gpsimd.collective_compute(
                    "AllReduce", ALU.add,
                    replica_groups=[list(range(NCORES))],
                    ins=[red_in.ap().opt()],
                    outs=[red_out.ap().opt()],
                )
            for hp in reversed(hist_pools):
                hp.__exit__(None, None, None)
            post_pools = (
                tc.tile_pool(name="post", bufs=2),
                tc.tile_pool(name="ppsum", bufs=2, space="PSUM"),
                tc.tile_pool(name="sc", bufs=1),
            )
            post = post_pools[0].__enter__()
            ppsum = post_pools[1].__enter__()
            scp = post_pools[2].__enter__()
            if debug_maps:
                nc.sync.dma_start(out_zw[:, :, :], red_out[0])
                nc.sync.dma_start(out_wm[:, :, :], red_out[1])

            # views of the reduced maps as two row-blocks per image
            # red_out[k, b] is [128, 512] = flat 65536 = [256, 256] row-major
            # row-block i (rows 128i..128i+127) = flat[128i*256 : ...] which is
            # partitions 64i..64i+63 ... NOT partition aligned. Use explicit AP:
            # flat index = p*512 + f ; row r = (p*512+f)//256 = 2p + (f>=256)
            # block0 rows 0..127 -> p in 0..63 both halves. So instead load via
            # DMA with a [128, 256] pattern from the DRAM buffer directly:
            # dram flat [65536]: row r starts at r*256.

            # scalar accumulators
            l1_part = scp.tile([128, 1], F32)
            nc.vector.memset(l1_part[:], 0.0)
            ms_part = scp.tile([128, 1], F32)
            nc.vector.memset(ms_part[:], 0.0)
            ss_part = scp.tile([128, 1], F32)
            nc.vector.memset(ss_part[:], 0.0)

            def bcast_cols(vals, name, n):
                """vals: [1, n] AP -> [128, n] via ones-matmul broadcast."""
                bc_ps = ppsum.tile([128, n], F32, tag=f"bc_ps_{name}")
                nc.tensor.matmul(bc_ps[:], ones1[:], vals, start=True, stop=True)
                bc = scp.tile([128, n], F32, tag=f"bc_{name}")
                nc.vector.tensor_copy(bc[:], bc_ps[:])
                return bc

            def img_minmax_has(blocks, vmask_blocks, name):
                mn_c = post.tile([128, 2], F32, tag="mm_mnc")
                mx_c = post.tile([128, 2], F32, tag="mm_mxc")
                hs_c = post.tile([128, 2], F32, tag="mm_hsc")
                for i, (d, m) in enumerate(zip(blocks, vmask_blocks)):
                    dm = post.tile([128, 256], F32, tag="mm_dm")
                    nc.vector.tensor_tensor(dm[:], d[:], m[:], ALU.mult)
                    fill = post.tile([128, 256], F32, tag="mm_fill")
                    nc.vector.tensor_scalar(fill[:], m[:], -1e30, 1e30, ALU.mult, ALU.add)
                    big = post.tile([128, 256], F32, tag="mm_big")
                    nc.vector.tensor_tensor(big[:], dm[:], fill[:], ALU.add)
                    nc.vector.tensor_reduce(mn_c[:, i:i + 1], big[:], AX.X, ALU.min)
                    fil2 = post.tile([128, 256], F32, tag="mm_fil2")
                    nc.vector.tensor_scalar(fil2[:], m[:], 1e30, -1e30, ALU.mult, ALU.add)
                    sml = post.tile([128, 256], F32, tag="mm_sml")
                    nc.vector.tensor_tensor(sml[:], dm[:], fil2[:], ALU.add)
                    nc.vector.tensor_reduce(mx_c[:, i:i + 1], sml[:], AX.X, ALU.max)
                    nc.vector.tensor_reduce(hs_c[:, i:i + 1], m[:], AX.X, ALU.max)
                nmn = post.tile([128, 2], F32, tag="mm_nmn")
                nc.vector.tensor_scalar(nmn[:], mn_c[:], -1.0, None, ALU.mult)
                mn1 = scp.tile([1, 2], F32, tag="mm_mn1")
                nc.gpsimd.tensor_reduce(mn1[:], nmn[:], AX.C, ALU.max)
                mx1 = scp.tile([1, 2], F32, tag="mm_mx1")
                nc.gpsimd.tensor_reduce(mx1[:], mx_c[:], AX.C, ALU.max)
                hs1 = scp.tile([1, 2], F32, tag="mm_hs1")
                nc.gpsimd.tensor_reduce(hs1[:], hs_c[:], AX.C, ALU.max)
                nvmin = scp.tile([1, 1], F32, tag="mm_nvmin")
                nc.vector.tensor_reduce(nvmin[:], mn1[:], AX.X, ALU.max)
                vmin = scp.tile([1, 1], F32, tag="mm_vmin")
                nc.vector.tensor_scalar(vmin[:], nvmin[:], -1.0, None, ALU.mult)
                vmax = scp.tile([1, 1], F32, tag="mm_vmax")
                nc.vector.tensor_reduce(vmax[:], mx1[:], AX.X, ALU.max)
                has = scp.tile([1, 1], F32, tag="mm_has")
                nc.vector.tensor_reduce(has[:], hs1[:], AX.X, ALU.max)
                return vmin, vmax, has

            def normalize_blocks(blocks, vmask_blocks, name):
                vmin, vmax, has = img_minmax_has(blocks, vmask_blocks, name)
                minv = scp.tile([1, 1], F32, tag="nm_minv")
                nc.vector.tensor_scalar(minv[:], vmin[:], MIN_DEPTH, None, ALU.max)
                maxv = scp.tile([1, 1], F32, tag="nm_maxv")
                nc.vector.tensor_scalar(maxv[:], vmax[:], MAX_DEPTH, None, ALU.min)
                # min_map = has*minv ; max_map = has*maxv + (1-has)*MAX_DEPTH
                minm = scp.tile([1, 1], F32, tag="nm_minm")
                nc.vector.tensor_tensor(minm[:], minv[:], has[:], ALU.mult)
                nhas = scp.tile([1, 1], F32, tag="nm_nhas")
                nc.vector.tensor_scalar(nhas[:], has[:], -1.0, 1.0, ALU.mult, ALU.add)
                t1 = scp.tile([1, 1], F32, tag="nm_t1")
                nc.vector.tensor_scalar(t1[:], nhas[:], MAX_DEPTH, None, ALU.mult)
                maxm = scp.tile([1, 1], F32, tag="nm_maxm")
                nc.vector.tensor_tensor(maxm[:], maxv[:], has[:], ALU.mult)
                nc.vector.tensor_tensor(maxm[:], maxm[:], t1[:], ALU.add)
                den = scp.tile([1, 1], F32, tag="nm_den")
                nc.vector.tensor_tensor(den[:], maxm[:], minm[:], ALU.subtract)
                nc.vector.tensor_scalar(den[:], den[:], 1e-8, None, ALU.add)
                rden = scp.tile([1, 1], F32, tag="nm_rden")
                scr1 = scp.tile([1, 1], F32, tag="nm_scr1")
                nc.vector.reciprocal_approx_accurate(rden[:], den[:], scr1[:])
                pair = scp.tile([1, 2], F32, tag="nm_pair")
                nc.vector.tensor_copy(pair[:, 0:1], minm[:])
                nc.vector.tensor_copy(pair[:, 1:2], rden[:])
                bc = bcast_cols(pair[:], "nm", 2)
                outs = []
                for i, (d, m) in enumerate(zip(blocks, vmask_blocks)):
                    nrm = post.tile([128, 256], F32, tag=f"nm_nrm{i}")
                    nc.vector.tensor_scalar(nrm[:], d[:], bc[:, 0:1], bc[:, 1:2],
                                            ALU.subtract, ALU.mult)
                    nc.vector.tensor_tensor(nrm[:], nrm[:], m[:], ALU.mult)
                    outs.append(nrm)
                return outs

            cZERO = cpool.tile([128, 256], F32, tag="cZERO")
            nc.vector.memset(cZERO[:], 0.0)

            def pool11(blocks, name):
                p0 = ppsum.tile([128, 256], F32, tag="pp0")
                nc.tensor.matmul(p0[:], Bv00[:], blocks[0][:], start=True, stop=False)
                nc.tensor.matmul(p0[:], Bv10[:], blocks[1][:], start=False, stop=True)
                p1 = ppsum.tile([128, 256], F32, tag="pp1")
                nc.tensor.matmul(p1[:], Bv01[:], blocks[0][:], start=True, stop=False)
                nc.tensor.matmul(p1[:], Bv00[:], blocks[1][:], start=False, stop=True)
                outs = []
                for i, p in enumerate((p0, p1)):
                    spad = post.tile([128, 268], F32, tag=f"pl_spad{i}")
                    nc.vector.memset(spad[:], 0.0)
                    nc.vector.tensor_tensor_scan(spad[:, 6:262], p[:], cZERO[:], 0.0,
                                                 ALU.add, ALU.add)
                    nc.vector.tensor_copy(spad[:, 262:268],
                                          spad[:, 261:262].broadcast_to([128, 6]))
                    pl = post.tile([128, 256], F32, tag=f"{name}_pl{i}", name=f"{name}_pl{i}_t")
                    nc.vector.tensor_tensor(pl[:], spad[:, 11:267], spad[:, 0:256],
                                            ALU.subtract)
                    nc.scalar.activation(pl[:], pl[:], ACTF.Copy, scale=1.0 / 121.0)
                    outs.append(pl)
                return outs

            for b in range(B):
                zwb, wmb, gtb = [], [], []
                for i in range(2):
                    zt_ = post.tile([128, 256], F32, tag=f"zwb{i}")
                    # rows 128i..128i+127 of image b: dram flat offset
                    # (k, b, p, f) view: row r = 2p + (f>=256). Simplest: view
                    # red_out[k, b] as [128, 512]; row-block i is NOT a sub-AP.
                    # Use a 3-level AP via rearrange on the dram tensor AP.
                    src = red_out[0, b].rearrange("p (h f) -> (p h) f", h=2)
                    nc.sync.dma_start(zt_[:], src[128 * i:128 * (i + 1), :])
                    zwb.append(zt_)
                    wt_ = post.tile([128, 256], F32, tag=f"wmb{i}")
                    srw = red_out[1, b].rearrange("p (h f) -> (p h) f", h=2)
                    nc.sync.dma_start(wt_[:], srw[128 * i:128 * (i + 1), :])
                    wmb.append(wt_)
                    gt_ = post.tile([128, 256], F32, tag=f"gtb{i}")
                    nc.sync.dma_start(gt_[:], gt_in[b, i])
                    gtb.append(gt_)
                dep, pmask, gmaskr = [], [], []
                for i in range(2):
                    wmc = post.tile([128, 256], F32, tag=f"wmc{i}")
                    nc.vector.tensor_scalar(wmc[:], wmb[i][:], 1e-30, None, ALU.max)
                    rw = post.tile([128, 256], F32, tag=f"rw{i}")
                    scr2 = post.tile([128, 256], F32, tag=f"pscr{i}")
                    nc.vector.reciprocal_approx_accurate(rw[:], wmc[:], scr2[:])
                    dp = post.tile([128, 256], F32, tag=f"dp{i}")
                    nc.vector.tensor_tensor(dp[:], zwb[i][:], rw[:], ALU.mult)
                    pm = post.tile([128, 256], F32, tag=f"pm{i}")
                    nc.vector.tensor_scalar(pm[:], wmb[i][:], 0.0, None, ALU.is_gt)
                    nc.vector.tensor_tensor(dp[:], dp[:], pm[:], ALU.mult)
                    gm = post.tile([128, 256], F32, tag=f"gm{i}")
                    nc.vector.tensor_scalar(gm[:], gtb[i][:], 0.0, None, ALU.is_gt)
                    dep.append(dp); pmask.append(pm); gmaskr.append(gm)
                pn = normalize_blocks(dep, pmask, f"pn{b}")
                gn = normalize_blocks(gtb, gmaskr, f"gn{b}")

                vm = []
                for i in range(2):
                    pmn = post.tile([128, 256], F32, tag=f"pmn{i}")
                    nc.vector.tensor_scalar(pmn[:], pn[i][:], 0.0, None, ALU.is_gt)
                    gmn = post.tile([128, 256], F32, tag=f"gmn{i}")
                    nc.vector.tensor_scalar(gmn[:], gn[i][:], 0.0, None, ALU.is_gt)
                    v = post.tile([128, 256], F32, tag=f"vmk{i}")
                    nc.vector.tensor_tensor(v[:], pmn[:], gmn[:], ALU.mult)
                    vm.append(v)

                for i in range(2):
                    df = post.tile([128, 256], F32, tag=f"df{i}")
                    nc.vector.tensor_tensor(df[:], pn[i][:], gn[i][:], ALU.subtract)
                    ab = post.tile([128, 256], F32, tag=f"ab{i}")
                    nc.scalar.activation(ab[:], df[:], ACTF.Abs)
                    l1a = post.tile([128, 1], F32, tag=f"l1a{i}")
                    nc.vector.scalar_tensor_tensor(ab[:], ab[:], 1.0, vm[i][:],
                                                   ALU.mult, ALU.mult, accum_out=l1a[:])
                    nc.vector.tensor_tensor(l1_part[:], l1_part[:], l1a[:], ALU.add)
                    msa = post.tile([128, 1], F32, tag=f"msa{i}")
                    nc.vector.tensor_scalar(vm[i][:], vm[i][:], 1.0, None, ALU.mult,
                                            ALU.add, accum_out=msa[:])
                    nc.vector.tensor_tensor(ms_part[:], ms_part[:], msa[:], ALU.add)

                p2 = [post.tile([128, 256], F32, tag=f"p2_{i}", name=f"p2_{b}_{i}") for i in range(2)]
                g2 = [post.tile([128, 256], F32, tag=f"g2_{i}", name=f"g2_{b}_{i}") for i in range(2)]
                pg = [post.tile([128, 256], F32, tag=f"pg_{i}", name=f"pg_{b}_{i}") for i in range(2)]
                for i in range(2):
                    nc.vector.tensor_tensor(p2[i][:], pn[i][:], pn[i][:], ALU.mult)
                    nc.vector.tensor_tensor(g2[i][:], gn[i][:], gn[i][:], ALU.mult)
                    nc.vector.tensor_tensor(pg[i][:], pn[i][:], gn[i][:], ALU.mult)
                mu1 = pool11(pn, "mu1")
                mu2 = pool11(gn, "mu2")
                ep2 = pool11(p2, "ep2")
                eg2 = pool11(g2, "eg2")
                epg = pool11(pg, "epg")
                for i in range(2):
                    m11 = post.tile([128, 256], F32, tag=f"m11_{i}")
                    nc.vector.tensor_tensor(m11[:], mu1[i][:], mu1[i][:], ALU.mult)
                    m22 = post.tile([128, 256], F32, tag=f"m22_{i}")
                    nc.vector.tensor_tensor(m22[:], mu2[i][:], mu2[i][:], ALU.mult)
                    m12 = post.tile([128, 256], F32, tag=f"m12_{i}")
                    nc.vector.tensor_tensor(m12[:], mu1[i][:], mu2[i][:], ALU.mult)
                    s1 = post.tile([128, 256], F32, tag=f"s1_{i}")
                    nc.vector.tensor_tensor(s1[:], ep2[i][:], m11[:], ALU.subtract)
                    s2 = post.tile([128, 256], F32, tag=f"s2_{i}")
                    nc.vector.tensor_tensor(s2[:], eg2[i][:], m22[:], ALU.subtract)
                    s12 = post.tile([128, 256], F32, tag=f"s12_{i}")
                    nc.vector.tensor_tensor(s12[:], epg[i][:], m12[:], ALU.subtract)
                    na = post.tile([128, 256], F32, tag=f"na_{i}")
                    nc.vector.tensor_scalar(na[:], m12[:], 2.0, SSIM_C1, ALU.mult, ALU.add)
                    nb = post.tile([128, 256], F32, tag=f"nb_{i}")
                    nc.vector.tensor_scalar(nb[:], s12[:], 2.0, SSIM_C2, ALU.mult, ALU.add)
                    num = post.tile([128, 256], F32, tag=f"num_{i}")
                    nc.vector.tensor_tensor(num[:], na[:], nb[:], ALU.mult)
                    da = post.tile([128, 256], F32, tag=f"da_{i}")
                    nc.vector.tensor_tensor(da[:], m11[:], m22[:], ALU.add)
                    nc.vector.tensor_scalar(da[:], da[:], SSIM_C1, None, ALU.add)
                    db = post.tile([128, 256], F32, tag=f"db_{i}")
                    nc.vector.tensor_tensor(db[:], s1[:], s2[:], ALU.add)
                    nc.vector.tensor_scalar(db[:], db[:], SSIM_C2, None, ALU.add)
                    dd = post.tile([128, 256], F32, tag=f"dd_{i}")
                    nc.vector.tensor_tensor(dd[:], da[:], db[:], ALU.mult)
                    rd = post.tile([128, 256], F32, tag=f"rd_{i}")
                    scr3 = post.tile([128, 256], F32, tag=f"sscr_{i}")
                    nc.vector.reciprocal_approx_accurate(rd[:], dd[:], scr3[:])
                    sm = post.tile([128, 256], F32, tag=f"sm_{i}")
                    nc.vector.tensor_tensor(sm[:], num[:], rd[:], ALU.mult)
                    ssa = post.tile([128, 1], F32, tag=f"ssa_{i}")
                    nc.vector.scalar_tensor_tensor(sm[:], sm[:], 1.0, vm[i][:],
                                                   ALU.mult, ALU.mult, accum_out=ssa[:])
                    nc.vector.tensor_tensor(ss_part[:], ss_part[:], ssa[:], ALU.add)

            # ---------- final scalar ----------
            sums3 = scp.tile([128, 3], F32)
            nc.vector.tensor_copy(sums3[:, 0:1], l1_part[:])
            nc.vector.tensor_copy(sums3[:, 1:2], ms_part[:])
            nc.vector.tensor_copy(sums3[:, 2:3], ss_part[:])
            sum1 = scp.tile([1, 3], F32)
            nc.gpsimd.tensor_reduce(sum1[:], sums3[:], AX.C, ALU.add)
            msd = scp.tile([1, 1], F32, tag="fs_msd")
            nc.vector.tensor_scalar(msd[:], sum1[:, 1:2], 1e-8, None, ALU.add)
            rms = scp.tile([1, 1], F32, tag="fs_rms")
            scr4 = scp.tile([1, 1], F32, tag="fs_scr4")
            nc.vector.reciprocal_approx_accurate(rms[:], msd[:], scr4[:])
            l1v = scp.tile([1, 1], F32, tag="fs_l1v")
            nc.vector.tensor_tensor(l1v[:], sum1[:, 0:1], rms[:], ALU.mult)
            sfr = scp.tile([1, 1], F32, tag="fs_sfr")
            nc.vector.tensor_tensor(sfr[:], sum1[:, 2:3], rms[:], ALU.mult)
            ssimv = scp.tile([1, 1], F32, tag="fs_ssimv")
            nc.vector.tensor_scalar(ssimv[:], sfr[:], -1.0, 1.0, ALU.mult, ALU.add)
            l1w = scp.tile([1, 1], F32, tag="fs_l1w")
            nc.vector.tensor_scalar(l1w[:], l1v[:], 0.8, None, ALU.mult)
            tot = scp.tile([1, 1], F32, tag="fs_tot")
            nc.vector.scalar_tensor_tensor(tot[:], ssimv[:], 0.2, l1w[:],
                                           ALU.mult, ALU.add)
            nc.vector.tensor_scalar(tot[:], tot[:], 1.0, None, ALU.min)
            gate = scp.tile([1, 1], F32, tag="fs_gate")
            nc.vector.tensor_scalar(gate[:], sum1[:, 1:2], 10.0, None, ALU.is_ge)
            nc.vector.tensor_tensor(tot[:], tot[:], gate[:], ALU.mult)
            nc.sync.dma_start(out_scalar[:, :], tot[:])
            for pp in reversed(post_pools):
                pp.__exit__(None, None, None)

    nc.compile()
    return nc


def shard_inputs(points, densities, depth_gt, C, T):
    """points [B,N,3], densities [B,N,1], depth_gt [B,1,256,256] ->
    per-core input dicts. Core c handles image c//2, point half c%2."""
    Bb, N, _ = points.shape
    npts = 128 * C * T
    gt_flat = np.ascontiguousarray(
        depth_gt.reshape(B, 2, 128, 256), dtype=np.float32)
    points = np.asarray(points)
    densities = np.asarray(densities)
    ins = []
    for c in range(NCORES):
        b, h = c // 2, c % 2
        lo_i = h * npts
        hi_i = min(N, (h + 1) * npts)
        n = max(0, hi_i - lo_i)
        x = np.zeros(npts, np.float32)
        y = np.zeros(npts, np.float32)
        z = np.zeros(npts, np.float32)  # z=0 -> invalid padding
        d = np.zeros(npts, np.float32)
        if n > 0:
            p = points[b, lo_i:lo_i + n]
            x[:n] = p[:, 0]
            y[:n] = p[:, 1]
            z[:n] = p[:, 2]
            d[:n] = densities[b, lo_i:lo_i + n, 0]
        msk = np.zeros((128, B), np.float32)
        msk[:, b] = 1.0
        ins.append(dict(
            x_in=x.reshape(128, C * T), y_in=y.reshape(128, C * T),
            z_in=z.reshape(128, C * T), d_in=d.reshape(128, C * T),
            gt_in=gt_flat, msk_in=msk,
        ))
    return ins


def run(points, densities, depth_gt, C=512, T=16, nc=None, debug_maps=False,
        **kw):
    if nc is None:
        nc = build_nc(C, T, debug_maps=debug_maps)
    ins = shard_inputs(points, densities, depth_gt, C, T)
    return run_bass_kernel_spmd(nc, ins, core_ids=list(range(NCORES)), **kw)


_NC_CACHE = {}


def kernel(points, densities, depth_gt):
    points = np.asarray(points, dtype=np.float32)
    densities = np.asarray(densities, dtype=np.float32)
    depth_gt = np.asarray(depth_gt, dtype=np.float32)
    C, T = 512, 16
    key = (C, T)
    if key not in _NC_CACHE:
        _NC_CACHE[key] = build_nc(C, T, debug_maps=False)
    res = run(points, densities, depth_gt, C, T, nc=_NC_CACHE[key])
    return np.float32(res.results[0]["out_scalar"].reshape(()))



# revision 3
# speedup vs baseline: 2436.3818x; 2436.3818x over previous
"""Trainium2 Bass kernel for DepthConsistencyLoss (compaction version).

kernel(points, densities, depth_gt) -> np.float32 scalar loss.

8-core SPMD, 2 cores per image, ~1M points per core. Only ~5.4% of
points are valid (in-frustum, z > 0.1), so each core first compacts the
valid points with a per-partition prefix-scan + gpsimd local_scatter
(4096 candidates -> <=320 slots per partition), then scatters just the
compacted points into [128,512] PSUM histograms via fp16 one-hot
matmuls. The per-image [2,128,512] zw/wm maps are AllReduce-summed
pairwise (the 2 cores sharing an image), every core computes the
divide/normalize/SSIM/L1 stages for its own image, and a tiny 8-core
AllReduce combines the scalar partials.
"""

import numpy as np

import concourse.bass as bass
import concourse.tile as tile
from concourse import bacc, bass_isa, mybir
from concourse.bass_utils import run_bass_kernel_spmd

F32 = mybir.dt.float32
F16 = mybir.dt.float16
I32 = mybir.dt.int32
I16 = mybir.dt.int16
ALU = mybir.AluOpType
ACTF = mybir.ActivationFunctionType
AX = mybir.AxisListType
ROP = bass_isa.ReduceOp

NCORES = 8
B = 4
H = W = 256
MIN_DEPTH = 0.1
MAX_DEPTH = 10.0
SSIM_C1 = 0.01 ** 2
SSIM_C2 = 0.03 ** 2

PC = 1024          # preprocessing sub-chunk width (columns)
NSUB = 8           # sub-chunks per core -> 128*PC*NSUB = 1,048,576 points
SPG = 4            # sub-chunks per scatter group
CAP = 320          # compacted capacity per partition per group (max seen 312)
U = 64             # histogram inner-loop unroll
NPTS = 128 * PC * NSUB


def build_nc(nsub=NSUB, spg=SPG, cap=CAP, debug_maps=False, num_devices=NCORES):
    NG = nsub // spg
    SC = spg * PC

    nc = bacc.Bacc("TRN2", target_bir_lowering=False, debug=False,
                   num_devices=num_devices)

    npts_cols = PC * nsub
    x_in = nc.dram_tensor("x_in", [128, npts_cols], F32, kind="ExternalInput")
    y_in = nc.dram_tensor("y_in", [128, npts_cols], F32, kind="ExternalInput")
    z_in = nc.dram_tensor("z_in", [128, npts_cols], F32, kind="ExternalInput")
    d_in = nc.dram_tensor("d_in", [128, npts_cols], F32, kind="ExternalInput")
    gt_in = nc.dram_tensor("gt_in", [2, 128, 256], F32, kind="ExternalInput")
    out_scalar = nc.dram_tensor("out_scalar", [1, 1], F32, kind="ExternalOutput")
    if debug_maps:
        out_zw = nc.dram_tensor("out_zw", [128, 512], F32, kind="ExternalOutput")
        out_wm = nc.dram_tensor("out_wm", [128, 512], F32, kind="ExternalOutput")

    red_in = nc.dram_tensor("red_in", [2, 128, 512], F32)
    red_out = nc.dram_tensor("red_out", [2, 128, 512], F32)
    red2_in = nc.dram_tensor("red2_in", [128, 4], F32)
    red2_out = nc.dram_tensor("red2_out", [128, 4], F32, addr_space="Shared")

    with tile.TileContext(nc) as tc:
        with (
            tc.tile_pool(name="const", bufs=1) as cpool,
            tc.tile_pool(name="psum", bufs=1, space="PSUM") as psum,
        ):
            # ---------- constants ----------
            io512i = cpool.tile([128, 512], I16)
            nc.gpsimd.iota(io512i[:], pattern=[[1, 512]], base=0, channel_multiplier=0)
            io512 = cpool.tile([128, 512], F16)
            nc.vector.tensor_copy(io512[:], io512i[:])
            io128i = cpool.tile([128, 128], I16)
            nc.gpsimd.iota(io128i[:], pattern=[[1, 128]], base=0, channel_multiplier=0)
            io128 = cpool.tile([128, 128], F16)
            nc.vector.tensor_copy(io128[:], io128i[:])

            # band matrices for the vertical 11-tap box filter (pool11)
            bands = {}
            for base in (0, 128, -128):
                di = cpool.tile([128, 128], I16, tag=f"bandi_{base}")
                nc.gpsimd.iota(di[:], pattern=[[1, 128]], base=base, channel_multiplier=-1)
                df = cpool.tile([128, 128], F32, tag=f"bandf_{base}")
                nc.vector.tensor_copy(df[:], di[:])
                c1t = cpool.tile([128, 128], F32, tag=f"bandc1_{base}")
                nc.vector.tensor_scalar(c1t[:], df[:], -5.0, None, ALU.is_ge)
                c2t = cpool.tile([128, 128], F32, tag=f"bandc2_{base}")
                nc.vector.tensor_scalar(c2t[:], df[:], 5.0, None, ALU.is_le)
                bt = cpool.tile([128, 128], F32, tag=f"band_{base}")
                nc.vector.tensor_tensor(bt[:], c1t[:], c2t[:], ALU.mult)
                bands[base] = bt
            Bv00, Bv01, Bv10 = bands[0], bands[128], bands[-128]

            zeros1k = cpool.tile([128, PC], F32, tag="zeros1k")
            nc.vector.memset(zeros1k[:], 0.0)
            b128 = cpool.tile([128, 1], F32, tag="b128")
            nc.vector.memset(b128[:], 128.0)
            bm1275 = cpool.tile([128, 1], F32, tag="bm1275")
            nc.vector.memset(bm1275[:], -127.5)
            bzt = cpool.tile([128, 1], F32, tag="bzt")
            nc.vector.memset(bzt[:], 127.5 + MIN_DEPTH * 1e6)
            b12775 = cpool.tile([128, 1], F32, tag="b12775")
            nc.vector.memset(b12775[:], 127.75)
            bm32768 = cpool.tile([128, 1], F32, tag="bm32768")
            nc.vector.memset(bm32768[:], -32768.0)
            cZERO = cpool.tile([128, 256], F32, tag="cZERO")
            nc.vector.memset(cZERO[:], 0.0)
            zlhs = cpool.tile([128, 128], F16, tag="zlhs")
            nc.vector.memset(zlhs[:], 0.0)
            zrhs = cpool.tile([128, 512], F16, tag="zrhs")
            nc.vector.memset(zrhs[:], 0.0)

            # ---------- histogram accumulators ----------
            acc_zw = psum.tile([128, 512], F32)
            acc_wm = psum.tile([128, 512], F32)
            nc.vector.memset(acc_zw[:], 0.0)
            nc.vector.memset(acc_wm[:], 0.0)

            hist_pools = (
                tc.tile_pool(name="ohp", bufs=5),
                tc.tile_pool(name="cmp", bufs=2),
                tc.tile_pool(name="pts", bufs=2),
                tc.tile_pool(name="work", bufs=1),
                tc.tile_pool(name="grp", bufs=2),
            )
            ohp = hist_pools[0].__enter__()
            cmp_ = hist_pools[1].__enter__()
            pts = hist_pools[2].__enter__()
            wk = hist_pools[3].__enter__()
            grp = hist_pools[4].__enter__()

            for g in range(NG):
                # group-level 2-byte staging (written slice-by-slice below)
                bin16g = grp.tile([128, SC], I16, tag="bin16g")
                w16g = grp.tile([128, SC], F16, tag="w16g")
                z16g = grp.tile([128, SC], F16, tag="z16g")
                dest16g = grp.tile([128, SC], I16, tag="dest16g")
                incl_prev = None
                for si in range(spg):
                    s = g * spg + si
                    sl = slice(s * PC, (s + 1) * PC)
                    gsl = slice(si * PC, (si + 1) * PC)
                    xt = pts.tile([128, PC], F32, tag="xt")
                    yt = pts.tile([128, PC], F32, tag="yt")
                    zt = pts.tile([128, PC], F32, tag="zt")
                    dt = pts.tile([128, PC], F32, tag="dt")
                    nc.sync.dma_start(xt[:], x_in[:, sl])
                    nc.sync.dma_start(yt[:], y_in[:, sl])
                    nc.sync.dma_start(zt[:], z_in[:, sl])
                    nc.sync.dma_start(dt[:], d_in[:, sl])

                    # -------- projection --------
                    zs = wk.tile([128, PC], F32, tag="zs")
                    nc.vector.tensor_scalar(zs[:], zt[:], MIN_DEPTH, None, ALU.max)
                    nc.gpsimd.tensor_copy(z16g[:, gsl], zs[:])
                    scr = wk.tile([128, PC], F32, tag="scr")
                    rz = wk.tile([128, PC], F32, tag="rz")
                    nc.vector.reciprocal_approx_accurate(rz[:], zs[:], scr[:])
                    u0 = wk.tile([128, PC], F32, tag="u0")
                    nc.vector.tensor_tensor(u0[:], xt[:], rz[:], ALU.mult)
                    v0 = wk.tile([128, PC], F32, tag="v0")
                    nc.vector.tensor_tensor(v0[:], yt[:], rz[:], ALU.mult)
                    sg = wk.tile([128, PC], F32, tag="sg")
                    nc.scalar.activation(sg[:], dt[:], ACTF.Sigmoid)
                    upx = wk.tile([128, PC], F32, tag="upx")
                    nc.scalar.activation(upx[:], u0[:], ACTF.Identity,
                                         bias=b128[:, 0:1], scale=256.0)
                    vpx = wk.tile([128, PC], F32, tag="vpx")
                    nc.scalar.activation(vpx[:], v0[:], ACTF.Identity,
                                         bias=b128[:, 0:1], scale=256.0)

                    # -------- floor via the 2^23 magic-number trick ----
                    # t = (x + 2^23) - 2^23 = rte-round(x); uf = t - (t > x)
                    MAGIC = 8388608.0
                    fru = wk.tile([128, PC], F32, tag="rf")
                    nc.vector.tensor_scalar(fru[:], upx[:], MAGIC, None, ALU.add)
                    nc.vector.tensor_scalar(fru[:], fru[:], MAGIC, None, ALU.subtract)
                    gg = wk.tile([128, PC], F32, tag="gg")
                    nc.vector.tensor_tensor(gg[:], fru[:], upx[:], ALU.is_gt)
                    nc.vector.tensor_tensor(upx[:], fru[:], gg[:], ALU.subtract)
                    nc.vector.tensor_scalar(fru[:], vpx[:], MAGIC, None, ALU.add)
                    nc.vector.tensor_scalar(fru[:], fru[:], MAGIC, None, ALU.subtract)
                    nc.vector.tensor_tensor(gg[:], fru[:], vpx[:], ALU.is_gt)
                    nc.vector.tensor_tensor(vpx[:], fru[:], gg[:], ALU.subtract)

                    # validity: 0 <= uf,vf <= 255 (exact on integers), z > 0.1
                    au = wk.tile([128, PC], F32, tag="u0", name="au")
                    nc.scalar.activation(au[:], upx[:], ACTF.Abs, bias=bm1275[:, 0:1])
                    av = wk.tile([128, PC], F32, tag="v0", name="av")
                    nc.scalar.activation(av[:], vpx[:], ACTF.Abs, bias=bm1275[:, 0:1])
                    zterm = wk.tile([128, PC], F32, tag="zs", name="zterm")
                    nc.scalar.activation(zterm[:], zt[:], ACTF.Identity,
                                         bias=bzt[:, 0:1], scale=-1e6)
                    mx = wk.tile([128, PC], F32, tag="mx")
                    nc.vector.tensor_tensor(mx[:], au[:], av[:], ALU.max)
                    nc.vector.tensor_tensor(mx[:], mx[:], zterm[:], ALU.max)
                    vsgn = wk.tile([128, PC], F32, tag="gg", name="vsgn")
                    nc.scalar.activation(vsgn[:], mx[:], ACTF.Sign,
                                         bias=b12775[:, 0:1], scale=-1.0)
                    valid = wk.tile([128, PC], F32, tag="valid")
                    nc.scalar.activation(valid[:], vsgn[:], ACTF.Relu)
                    nc.gpsimd.tensor_tensor(w16g[:, gsl], valid[:], sg[:], ALU.mult)

                    # bin - 32768 as f32, then one cast to i16
                    vpm = wk.tile([128, PC], F32, tag="rf", name="vpm")
                    nc.scalar.activation(vpm[:], vpx[:], ACTF.Identity,
                                         bias=bm32768[:, 0:1], scale=256.0)
                    binfm = wk.tile([128, PC], F32, tag="u0", name="binfm")
                    nc.vector.tensor_tensor(binfm[:], vpm[:], upx[:], ALU.add)
                    nc.gpsimd.tensor_copy(bin16g[:, gsl], binfm[:])

                    # -------- compaction destinations --------
                    incl = wk.tile([128, PC], F32, tag=f"incl{si % 2}")
                    init = 0.0 if si == 0 else incl_prev[:, PC - 1:PC]
                    nc.vector.tensor_tensor_scan(incl[:], valid[:], zeros1k[:],
                                                 init, ALU.add, ALU.add)
                    incl_prev = incl
                    t2a = wk.tile([128, PC], F32, tag="v0", name="t2a")
                    nc.vector.scalar_tensor_tensor(t2a[:], incl[:], float(cap),
                                                   valid[:], ALU.is_le, ALU.mult)
                    nc.vector.tensor_tensor(t2a[:], incl[:], t2a[:], ALU.mult)
                    nc.vector.tensor_scalar(t2a[:], t2a[:], 1.0, None, ALU.subtract)
                    nc.gpsimd.tensor_copy(dest16g[:, gsl], t2a[:])

                # -------- compact via gpsimd local_scatter --------
                cbin = cmp_.tile([128, cap], I16, tag="cbin")
                cw = cmp_.tile([128, cap], F16, tag="cw")
                cz = cmp_.tile([128, cap], F16, tag="cz")
                nc.gpsimd.local_scatter(cbin[:], bin16g[:], dest16g[:], 128, cap, SC)
                nc.gpsimd.local_scatter(cw[:], w16g[:], dest16g[:], 128, cap, SC)
                nc.gpsimd.local_scatter(cz[:], z16g[:], dest16g[:], 128, cap, SC)

                # -------- derive per-point scalars --------
                bini2 = cmp_.tile([128, cap], I32, tag="bini2")
                nc.gpsimd.tensor_copy(bini2[:], cbin[:])
                nc.gpsimd.tensor_scalar(bini2[:], bini2[:], 32768, None, ALU.add)
                hifi = cmp_.tile([128, cap], I32, tag="hifi")
                nc.vector.tensor_scalar(hifi[:], bini2[:], 9, None,
                                        ALU.arith_shift_right)
                hiff = cmp_.tile([128, cap], F32, tag="hiff")
                nc.gpsimd.tensor_copy(hiff[:], hifi[:])
                lofi = cmp_.tile([128, cap], I32, tag="lofi")
                nc.vector.tensor_scalar(lofi[:], bini2[:], 511, None,
                                        ALU.bitwise_and)
                loff = cmp_.tile([128, cap], F32, tag="loff")
                nc.gpsimd.tensor_copy(loff[:], lofi[:])
                wf = cmp_.tile([128, cap], F32, tag="wf")
                nc.gpsimd.tensor_copy(wf[:], cw[:])
                zf = cmp_.tile([128, cap], F32, tag="zf")
                nc.gpsimd.tensor_copy(zf[:], cz[:])

                # -------- one-hot matmul histogram --------
                with tc.For_i(0, cap, U) as iv:
                    lofb = ohp.tile([128, U], F32, tag="lofb")
                    nc.vector.tensor_copy(lofb[:], loff[:, bass.ds(iv, U)])
                    wb = ohp.tile([128, U], F32, tag="wb")
                    nc.vector.tensor_copy(wb[:], wf[:, bass.ds(iv, U)])
                    hifb = ohp.tile([128, U], F32, tag="hifb")
                    nc.vector.tensor_copy(hifb[:], hiff[:, bass.ds(iv, U)])
                    zb = ohp.tile([128, U], F32, tag="zb")
                    nc.vector.tensor_copy(zb[:], zf[:, bass.ds(iv, U)])
                    for u in range(U):
                        rhs = ohp.tile([128, 512], F16, tag="rhs")
                        nc.vector.tensor_scalar(rhs[:], io512[:], lofb[:, u:u + 1],
                                                wb[:, u:u + 1], ALU.is_equal, ALU.mult)
                        lhsA = ohp.tile([128, 128], F16, tag="lhsA")
                        nc.vector.tensor_scalar(lhsA[:], io128[:], hifb[:, u:u + 1],
                                                None, ALU.is_equal)
                        lhsZ = ohp.tile([128, 128], F16, tag="lhsZ")
                        nc.scalar.activation(lhsZ[:], lhsA[:], ACTF.Copy,
                                             scale=zb[:, u:u + 1])
                        nc.tensor.matmul(acc_wm[:], lhsA[:], rhs[:], start=False,
                                         stop=False, skip_group_check=True)
                        nc.tensor.matmul(acc_zw[:], lhsZ[:], rhs[:], start=False,
                                         stop=False, skip_group_check=True)

            nc.tensor.matmul(acc_wm[:], zlhs[:], zrhs[:], start=False, stop=True,
                             skip_group_check=True)
            nc.tensor.matmul(acc_zw[:], zlhs[:], zrhs[:], start=False, stop=True,
                             skip_group_check=True)

            # ---------- pairwise AllReduce of the image maps ----------
            sb_zw = cpool.tile([128, 512], F32, tag="sb_zw")
            nc.vector.tensor_copy(sb_zw[:], acc_zw[:])
            sb_wm = cpool.tile([128, 512], F32, tag="sb_wm")
            nc.vector.tensor_copy(sb_wm[:], acc_wm[:])
            nc.sync.dma_start(red_in[0], sb_zw[:])
            nc.sync.dma_start(red_in[1], sb_wm[:])
            nc.gpsimd.collective_compute(
                "AllReduce", ALU.add,
                replica_groups=[[0, 1], [2, 3], [4, 5], [6, 7]],
                ins=[red_in.ap().opt()],
                outs=[red_out.ap().opt()],
            )
            for hp in reversed(hist_pools):
                hp.__exit__(None, None, None)

            post_pools = (
                tc.tile_pool(name="post", bufs=2),
                tc.tile_pool(name="ppsum", bufs=2, space="PSUM"),
                tc.tile_pool(name="sc", bufs=1),
            )
            post = post_pools[0].__enter__()
            ppsum = post_pools[1].__enter__()
            scp = post_pools[2].__enter__()
            if debug_maps:
                nc.sync.dma_start(out_zw[:, :], red_out[0])
                nc.sync.dma_start(out_wm[:, :], red_out[1])

            # scalar accumulators (per-partition)
            l1_part = scp.tile([128, 1], F32)
            nc.vector.memset(l1_part[:], 0.0)
            ms_part = scp.tile([128, 1], F32)
            nc.vector.memset(ms_part[:], 0.0)
            ss_part = scp.tile([128, 1], F32)
            nc.vector.memset(ss_part[:], 0.0)

            def img_minmax_has(blocks, vmask_blocks):
                """Returns vmin, vmax, has as [128,1] tiles (same value in
                every partition)."""
                mn_c = post.tile([128, 2], F32, tag="mm_mnc")
                mx_c = post.tile([128, 2], F32, tag="mm_mxc")
                hs_c = post.tile([128, 2], F32, tag="mm_hsc")
                for i, (d, m) in enumerate(zip(blocks, vmask_blocks)):
                    dm = post.tile([128, 256], F32, tag="mm_dm")
                    nc.vector.tensor_tensor(dm[:], d[:], m[:], ALU.mult)
                    fill = post.tile([128, 256], F32, tag="mm_fill")
                    nc.vector.tensor_scalar(fill[:], m[:], -1e30, 1e30, ALU.mult, ALU.add)
                    big = post.tile([128, 256], F32, tag="mm_big")
                    nc.vector.tensor_tensor(big[:], dm[:], fill[:], ALU.add)
                    nc.vector.tensor_reduce(mn_c[:, i:i + 1], big[:], AX.X, ALU.min)
                    fil2 = post.tile([128, 256], F32, tag="mm_fil2")
                    nc.vector.tensor_scalar(fil2[:], m[:], 1e30, -1e30, ALU.mult, ALU.add)
                    sml = post.tile([128, 256], F32, tag="mm_sml")
                    nc.vector.tensor_tensor(sml[:], dm[:], fil2[:], ALU.add)
                    nc.vector.tensor_reduce(mx_c[:, i:i + 1], sml[:], AX.X, ALU.max)
                    nc.vector.tensor_reduce(hs_c[:, i:i + 1], m[:], AX.X, ALU.max)
                nmn = post.tile([128, 2], F32, tag="mm_nmn")
                nc.vector.tensor_scalar(nmn[:], mn_c[:], -1.0, None, ALU.mult)
                nmn_ar = post.tile([128, 2], F32, tag="mm_nmn_ar")
                nc.gpsimd.partition_all_reduce(nmn_ar[:], nmn[:], 128, ROP.max)
                mx_ar = post.tile([128, 2], F32, tag="mm_mx_ar")
                nc.gpsimd.partition_all_reduce(mx_ar[:], mx_c[:], 128, ROP.max)
                hs_ar = post.tile([128, 2], F32, tag="mm_hs_ar")
                nc.gpsimd.partition_all_reduce(hs_ar[:], hs_c[:], 128, ROP.max)
                nvmin = scp.tile([128, 1], F32, tag="mm_nvmin")
                nc.vector.tensor_reduce(nvmin[:], nmn_ar[:], AX.X, ALU.max)
                vmin = scp.tile([128, 1], F32, tag="mm_vmin")
                nc.vector.tensor_scalar(vmin[:], nvmin[:], -1.0, None, ALU.mult)
                vmax = scp.tile([128, 1], F32, tag="mm_vmax")
                nc.vector.tensor_reduce(vmax[:], mx_ar[:], AX.X, ALU.max)
                has = scp.tile([128, 1], F32, tag="mm_has")
                nc.vector.tensor_reduce(has[:], hs_ar[:], AX.X, ALU.max)
                return vmin, vmax, has

            def normalize_blocks(blocks, vmask_blocks, name):
                vmin, vmax, has = img_minmax_has(blocks, vmask_blocks)
                minv = scp.tile([128, 1], F32, tag="nm_minv")
                nc.vector.tensor_scalar(minv[:], vmin[:], MIN_DEPTH, None, ALU.max)
                maxv = scp.tile([128, 1], F32, tag="nm_maxv")
                nc.vector.tensor_scalar(maxv[:], vmax[:], MAX_DEPTH, None, ALU.min)
                minm = scp.tile([128, 1], F32, tag="nm_minm")
                nc.vector.tensor_tensor(minm[:], minv[:], has[:], ALU.mult)
                nhas = scp.tile([128, 1], F32, tag="nm_nhas")
                nc.vector.tensor_scalar(nhas[:], has[:], -1.0, 1.0, ALU.mult, ALU.add)
                t1 = scp.tile([128, 1], F32, tag="nm_t1")
                nc.vector.tensor_scalar(t1[:], nhas[:], MAX_DEPTH, None, ALU.mult)
                maxm = scp.tile([128, 1], F32, tag="nm_maxm")
                nc.vector.tensor_tensor(maxm[:], maxv[:], has[:], ALU.mult)
                nc.vector.tensor_tensor(maxm[:], maxm[:], t1[:], ALU.add)
                den = scp.tile([128, 1], F32, tag="nm_den")
                nc.vector.tensor_tensor(den[:], maxm[:], minm[:], ALU.subtract)
                nc.vector.tensor_scalar(den[:], den[:], 1e-8, None, ALU.add)
                rden = scp.tile([128, 1], F32, tag="nm_rden")
                scr1 = scp.tile([128, 1], F32, tag="nm_scr1")
                nc.vector.reciprocal_approx_accurate(rden[:], den[:], scr1[:])
                outs = []
                for i, (d, m) in enumerate(zip(blocks, vmask_blocks)):
                    nrm = post.tile([128, 256], F32, tag=f"{name}_nrm{i}",
                                    name=f"{name}_nrm{i}")
                    nc.vector.tensor_scalar(nrm[:], d[:], minm[:, 0:1], rden[:, 0:1],
                                            ALU.subtract, ALU.mult)
                    nc.vector.tensor_tensor(nrm[:], nrm[:], m[:], ALU.mult)
                    outs.append(nrm)
                return outs

            def pool11(blocks, name):
                p0 = ppsum.tile([128, 256], F32, tag="pp0")
                nc.tensor.matmul(p0[:], Bv00[:], blocks[0][:], start=True, stop=False)
                nc.tensor.matmul(p0[:], Bv10[:], blocks[1][:], start=False, stop=True)
                p1 = ppsum.tile([128, 256], F32, tag="pp1")
                nc.tensor.matmul(p1[:], Bv01[:], blocks[0][:], start=True, stop=False)
                nc.tensor.matmul(p1[:], Bv00[:], blocks[1][:], start=False, stop=True)
                outs = []
                for i, p in enumerate((p0, p1)):
                    spad = post.tile([128, 268], F32, tag=f"pl_spad{i}")
                    nc.vector.memset(spad[:], 0.0)
                    nc.vector.tensor_tensor_scan(spad[:, 6:262], p[:], cZERO[:], 0.0,
                                                 ALU.add, ALU.add)
                    nc.vector.tensor_copy(spad[:, 262:268],
                                          spad[:, 261:262].broadcast_to([128, 6]))
                    pl = post.tile([128, 256], F32, tag=f"{name}_pl{i}",
                                   name=f"{name}_pl{i}_t")
                    nc.vector.tensor_tensor(pl[:], spad[:, 11:267], spad[:, 0:256],
                                            ALU.subtract)
                    nc.scalar.activation(pl[:], pl[:], ACTF.Copy, scale=1.0 / 121.0)
                    outs.append(pl)
                return outs

            # ---------- post: own image only ----------
            zwb, wmb, gtb = [], [], []
            for i in range(2):
                zt_ = post.tile([128, 256], F32, tag=f"zwb{i}")
                src = red_out[0].rearrange("p (h f) -> (p h) f", h=2)
                nc.sync.dma_start(zt_[:], src[128 * i:128 * (i + 1), :])
                zwb.append(zt_)
                wt_ = post.tile([128, 256], F32, tag=f"wmb{i}")
                srw = red_out[1].rearrange("p (h f) -> (p h) f", h=2)
                nc.sync.dma_start(wt_[:], srw[128 * i:128 * (i + 1), :])
                wmb.append(wt_)
                gt_ = post.tile([128, 256], F32, tag=f"gtb{i}")
                nc.sync.dma_start(gt_[:], gt_in[i])
                gtb.append(gt_)
            dep, pmask, gmaskr = [], [], []
            for i in range(2):
                wmc = post.tile([128, 256], F32, tag=f"wmc{i}")
                nc.vector.tensor_scalar(wmc[:], wmb[i][:], 1e-30, None, ALU.max)
                rw = post.tile([128, 256], F32, tag=f"rw{i}")
                scr2 = post.tile([128, 256], F32, tag=f"pscr{i}")
                nc.vector.reciprocal_approx_accurate(rw[:], wmc[:], scr2[:])
                dp = post.tile([128, 256], F32, tag=f"dp{i}")
                nc.vector.tensor_tensor(dp[:], zwb[i][:], rw[:], ALU.mult)
                pm = post.tile([128, 256], F32, tag=f"pm{i}")
                nc.vector.tensor_scalar(pm[:], wmb[i][:], 0.0, None, ALU.is_gt)
                nc.vector.tensor_tensor(dp[:], dp[:], pm[:], ALU.mult)
                gm = post.tile([128, 256], F32, tag=f"gm{i}")
                nc.vector.tensor_scalar(gm[:], gtb[i][:], 0.0, None, ALU.is_gt)
                dep.append(dp); pmask.append(pm); gmaskr.append(gm)
            pn = normalize_blocks(dep, pmask, "pn")
            gn = normalize_blocks(gtb, gmaskr, "gn")

            vm = []
            for i in range(2):
                pmn = post.tile([128, 256], F32, tag=f"pmn{i}")
                nc.vector.tensor_scalar(pmn[:], pn[i][:], 0.0, None, ALU.is_gt)
                gmn = post.tile([128, 256], F32, tag=f"gmn{i}")
                nc.vector.tensor_scalar(gmn[:], gn[i][:], 0.0, None, ALU.is_gt)
                v = post.tile([128, 256], F32, tag=f"vmk{i}")
                nc.vector.tensor_tensor(v[:], pmn[:], gmn[:], ALU.mult)
                vm.append(v)

            for i in range(2):
                df = post.tile([128, 256], F32, tag=f"df{i}")
                nc.vector.tensor_tensor(df[:], pn[i][:], gn[i][:], ALU.subtract)
                ab = post.tile([128, 256], F32, tag=f"ab{i}")
                nc.scalar.activation(ab[:], df[:], ACTF.Abs)
                l1a = post.tile([128, 1], F32, tag=f"l1a{i}")
                nc.vector.scalar_tensor_tensor(ab[:], ab[:], 1.0, vm[i][:],
                                               ALU.mult, ALU.mult, accum_out=l1a[:])
                nc.vector.tensor_tensor(l1_part[:], l1_part[:], l1a[:], ALU.add)
                msa = post.tile([128, 1], F32, tag=f"msa{i}")
                nc.vector.tensor_scalar(vm[i][:], vm[i][:], 1.0, None, ALU.mult,
                                        ALU.add, accum_out=msa[:])
                nc.vector.tensor_tensor(ms_part[:], ms_part[:], msa[:], ALU.add)

            p2 = [post.tile([128, 256], F32, tag=f"p2_{i}", name=f"p2_{i}") for i in range(2)]
            g2 = [post.tile([128, 256], F32, tag=f"g2_{i}", name=f"g2_{i}") for i in range(2)]
            pg = [post.tile([128, 256], F32, tag=f"pg_{i}", name=f"pg_{i}") for i in range(2)]
            for i in range(2):
                nc.vector.tensor_tensor(p2[i][:], pn[i][:], pn[i][:], ALU.mult)
                nc.vector.tensor_tensor(g2[i][:], gn[i][:], gn[i][:], ALU.mult)
                nc.vector.tensor_tensor(pg[i][:], pn[i][:], gn[i][:], ALU.mult)
            mu1 = pool11(pn, "mu1")
            mu2 = pool11(gn, "mu2")
            ep2 = pool11(p2, "ep2")
            eg2 = pool11(g2, "eg2")
            epg = pool11(pg, "epg")
            for i in range(2):
                m11 = post.tile([128, 256], F32, tag=f"m11_{i}")
                nc.vector.tensor_tensor(m11[:], mu1[i][:], mu1[i][:], ALU.mult)
                m22 = post.tile([128, 256], F32, tag=f"m22_{i}")
                nc.vector.tensor_tensor(m22[:], mu2[i][:], mu2[i][:], ALU.mult)
                m12 = post.tile([128, 256], F32, tag=f"m12_{i}")
                nc.vector.tensor_tensor(m12[:], mu1[i][:], mu2[i][:], ALU.mult)
                s1 = post.tile([128, 256], F32, tag=f"s1_{i}")
                nc.vector.tensor_tensor(s1[:], ep2[i][:], m11[:], ALU.subtract)
                s2 = post.tile([128, 256], F32, tag=f"s2_{i}")
                nc.vector.tensor_tensor(s2[:], eg2[i][:], m22[:], ALU.subtract)
                s12 = post.tile([128, 256], F32, tag=f"s12_{i}")
                nc.vector.tensor_tensor(s12[:], epg[i][:], m12[:], ALU.subtract)
                na = post.tile([128, 256], F32, tag=f"na_{i}")
                nc.vector.tensor_scalar(na[:], m12[:], 2.0, SSIM_C1, ALU.mult, ALU.add)
                nb = post.tile([128, 256], F32, tag=f"nb_{i}")
                nc.vector.tensor_scalar(nb[:], s12[:], 2.0, SSIM_C2, ALU.mult, ALU.add)
                num = post.tile([128, 256], F32, tag=f"num_{i}")
                nc.vector.tensor_tensor(num[:], na[:], nb[:], ALU.mult)
                da = post.tile([128, 256], F32, tag=f"da_{i}")
                nc.vector.tensor_tensor(da[:], m11[:], m22[:], ALU.add)
                nc.vector.tensor_scalar(da[:], da[:], SSIM_C1, None, ALU.add)
                db = post.tile([128, 256], F32, tag=f"db_{i}")
                nc.vector.tensor_tensor(db[:], s1[:], s2[:], ALU.add)
                nc.vector.tensor_scalar(db[:], db[:], SSIM_C2, None, ALU.add)
                dd = post.tile([128, 256], F32, tag=f"dd_{i}")
                nc.vector.tensor_tensor(dd[:], da[:], db[:], ALU.mult)
                rd = post.tile([128, 256], F32, tag=f"rd_{i}")
                scr3 = post.tile([128, 256], F32, tag=f"sscr_{i}")
                nc.vector.reciprocal_approx_accurate(rd[:], dd[:], scr3[:])
                sm = post.tile([128, 256], F32, tag=f"sm_{i}")
                nc.vector.tensor_tensor(sm[:], num[:], rd[:], ALU.mult)
                ssa = post.tile([128, 1], F32, tag=f"ssa_{i}")
                nc.vector.scalar_tensor_tensor(sm[:], sm[:], 1.0, vm[i][:],
                                               ALU.mult, ALU.mult, accum_out=ssa[:])
                nc.vector.tensor_tensor(ss_part[:], ss_part[:], ssa[:], ALU.add)

            # ---------- global partial-sum AllReduce ----------
            pk = scp.tile([128, 4], F32, tag="pk")
            nc.vector.memset(pk[:], 0.0)
            nc.vector.tensor_scalar(pk[:, 0:1], l1_part[:], 0.5, None, ALU.mult)
            nc.vector.tensor_scalar(pk[:, 1:2], ms_part[:], 0.5, None, ALU.mult)
            nc.vector.tensor_scalar(pk[:, 2:3], ss_part[:], 0.5, None, ALU.mult)
            nc.sync.dma_start(red2_in[:, :], pk[:])
            nc.gpsimd.collective_compute(
                "AllReduce", ALU.add,
                replica_groups=[list(range(NCORES))],
                ins=[red2_in.ap().opt()],
                outs=[red2_out.ap().opt()],
            )
            sums = scp.tile([128, 4], F32, tag="sums")
            nc.sync.dma_start(sums[:], red2_out[:, :])
            tot4 = scp.tile([128, 4], F32, tag="tot4")
            nc.gpsimd.partition_all_reduce(tot4[:], sums[:], 128, ROP.add)

            # ---------- final scalar ----------
            sum1 = tot4
            msd = scp.tile([1, 1], F32, tag="fs_msd")
            nc.vector.tensor_scalar(msd[:], sum1[0:1, 1:2], 1e-8, None, ALU.add)
            rms = scp.tile([1, 1], F32, tag="fs_rms")
            scr4 = scp.tile([1, 1], F32, tag="fs_scr4")
            nc.vector.reciprocal_approx_accurate(rms[:], msd[:], scr4[:])
            l1v = scp.tile([1, 1], F32, tag="fs_l1v")
            nc.vector.tensor_tensor(l1v[:], sum1[0:1, 0:1], rms[:], ALU.mult)
            sfr = scp.tile([1, 1], F32, tag="fs_sfr")
            nc.vector.tensor_tensor(sfr[:], sum1[0:1, 2:3], rms[:], ALU.mult)
            ssimv = scp.tile([1, 1], F32, tag="fs_ssimv")
            nc.vector.tensor_scalar(ssimv[:], sfr[:], -1.0, 1.0, ALU.mult, ALU.add)
            l1w = scp.tile([1, 1], F32, tag="fs_l1w")
            nc.vector.tensor_scalar(l1w[:], l1v[:], 0.8, None, ALU.mult)
            tot = scp.tile([1, 1], F32, tag="fs_tot")
            nc.vector.scalar_tensor_tensor(tot[:], ssimv[:], 0.2, l1w[:],
                                           ALU.mult, ALU.add)
            nc.vector.tensor_scalar(tot[:], tot[:], 1.0, None, ALU.min)
            gate = scp.tile([1, 1], F32, tag="fs_gate")
            nc.vector.tensor_scalar(gate[:], sum1[0:1, 1:2], 10.0, None, ALU.is_ge)
            nc.vector.tensor_tensor(tot[:], tot[:], gate[:], ALU.mult)
            nc.sync.dma_start(out_scalar[:, :], tot[:])
            for pp in reversed(post_pools):
                pp.__exit__(None, None, None)

    nc.compile()
    return nc


def shard_inputs(points, densities, depth_gt, nsub=NSUB):
    """Core c handles image c//2, point half c%2."""
    points = np.asarray(points)
    densities = np.asarray(densities)
    depth_gt = np.asarray(depth_gt)
    Bb, N, _ = points.shape
    npts = 128 * PC * nsub
    gt_flat = np.ascontiguousarray(
        depth_gt.reshape(B, 2, 128, 256), dtype=np.float32)
    ins = []
    for c in range(NCORES):
        b, hh = c // 2, c % 2
        lo_i = hh * npts
        hi_i = min(N, (hh + 1) * npts)
        n = max(0, hi_i - lo_i)
        x = np.zeros(npts, np.float32)
        y = np.zeros(npts, np.float32)
        z = np.zeros(npts, np.float32)  # z=0 -> invalid padding
        d = np.zeros(npts, np.float32)
        if n > 0:
            p = points[b, lo_i:lo_i + n]
            x[:n] = p[:, 0]
            y[:n] = p[:, 1]
            z[:n] = p[:, 2]
            d[:n] = densities[b, lo_i:lo_i + n, 0]
        ins.append(dict(
            x_in=x.reshape(128, PC * nsub), y_in=y.reshape(128, PC * nsub),
            z_in=z.reshape(128, PC * nsub), d_in=d.reshape(128, PC * nsub),
            gt_in=gt_flat[b],
        ))
    return ins


_NC_CACHE = {}


def get_nc():
    key = (NSUB, SPG, CAP)
    if key not in _NC_CACHE:
        _NC_CACHE[key] = build_nc(NSUB, SPG, CAP)
    return _NC_CACHE[key]


def kernel(points, densities, depth_gt):
    points = np.asarray(points, dtype=np.float32)
    densities = np.asarray(densities, dtype=np.float32)
    depth_gt = np.asarray(depth_gt, dtype=np.float32)
    nc = get_nc()
    ins = shard_inputs(points, densities, depth_gt)
    res = run_bass_kernel_spmd(nc, ins, core_ids=list(range(NCORES)))
    return np.float32(res.results[0]["out_scalar"].reshape(()))
